# revision 31
# baseline (speedup 1.0000x reference)
"""AdaptiveRelativeAttn distributed Trainium2 kernel.

Sharding: tensor-parallel over the 8 heads (core h owns head h for both
batches).  Each core synthesizes its head's factor-conditioned weight slices
on-chip (contracting factor_size=32 on DVE), computes q/k/v/r projections,
runs causal relative attention (Transformer-XL rel-shift realized as a single
diagonal-access-pattern DMA per key tile), and applies its head's slice of
the output projection, producing a partial [T*B, D] output.  Host sums the 8
partials (the out-proj head contraction) and reshapes.

Precision: the score path (weights, q/k/r, AC/BD, softmax input) is f32 —
scores here have sigma~22 so softmax is near-argmax and bf16 scores flip
argmaxes.  Post-softmax (probs, v, ctx, out-proj) is bf16.
"""

import sys

import numpy as np

if "/opt/trn_rl_repo" not in sys.path:
    sys.path.insert(0, "/opt/trn_rl_repo")

import ml_dtypes

import concourse.bass as bass
import concourse.mybir as mybir
import concourse.tile as tile
from concourse.bass_utils import run_bass_kernel_spmd

# Problem constants (hardcoded per harness contract).
D = 512
H = 8
DH = 64          # head dim
F = 32           # factor size
T = 2048
B = 2
SCALE = 1.0 / np.sqrt(DH)  # 0.125

BF16 = mybir.dt.bfloat16
F32 = mybir.dt.float32
AF = mybir.ActivationFunctionType
ALU = mybir.AluOpType

NEG_BIG = -1.0e9
DBG_BLK = 2




def _prune_dma_waits(nc, max_waits=2):
    """Walrus's DMA_DIRECT2D encoding only fits ``max_waits`` sync waits.
    Tile's per-proc wait assignment can emit more (cross-queue slot-reuse
    deps).  Drop waits that are transitively implied by the remaining ones:
    wait (S>=v) is implied if another wait's producer instruction already
    had knowledge of S>=v at its completion (procs complete in order)."""
    insts = []
    for f in nc.m.functions:
        for bb in f.blocks:
            insts.extend(bb.instructions)

    def proc_key(inst):
        p = inst.bass_scheduled_proc
        if p is not None:
            return f"P{p}"
        e = inst.engine
        return f"E{e.name}" if e is not None else "E?"

    import bisect
    from collections import defaultdict

    cum = defaultdict(int)          # sem id -> cumulative value
    sem_points = defaultdict(list)  # sem id -> [(cumvalue, idx)]
    know = [None] * len(insts)      # idx -> {sem id: value known}
    last_on_proc = {}

    def join(dst, src):
        for k, v in src.items():
            if dst.get(k, -1) < v:
                dst[k] = v

    for idx, inst in enumerate(insts):
        k = {}
        pk = proc_key(inst)
        if pk in last_on_proc:
            join(k, know[last_on_proc[pk]])
        si = inst.sync_info
        if si and si.on_wait:
            for w in si.on_wait:
                if w.wait_mode != "sem-ge-imm":
                    continue
                pts = sem_points.get(w.id)
                if pts:
                    pos = bisect.bisect_left(pts, (w.wait_value, -1))
                    if pos < len(pts):
                        join(k, know[pts[pos][1]])
                if k.get(w.id, -1) < w.wait_value:
                    k[w.id] = w.wait_value
        if si and si.on_update:
            for u in si.on_update:
                if u.update_mode == "sem-inc":
                    cum[u.id] += u.update_value
                    sem_points[u.id].append((cum[u.id], idx))
                    if k.get(u.id, -1) < cum[u.id]:
                        k[u.id] = cum[u.id]
        know[idx] = k
        last_on_proc[pk] = idx

    n_pruned = 0
    for idx, inst in enumerate(insts):
        tn = type(inst).__name__
        if not ("DMA" in tn or "Dma" in tn):
            continue
        si = inst.sync_info
        if not si or not si.on_wait or len(si.on_wait) <= max_waits:
            continue
        waits = list(si.on_wait)
        kept = list(waits)
        for w in waits:
            if len(kept) <= max_waits:
                break
            if w.wait_mode != "sem-ge-imm":
                continue
            others = [o for o in kept if o is not w]
            implied = False
            for o in others:
                if o.wait_mode != "sem-ge-imm":
                    continue
                pts = sem_points.get(o.id)
                if not pts:
                    continue
                pos = bisect.bisect_left(pts, (o.wait_value, -1))
                if pos < len(pts):
                    pk = know[pts[pos][1]]
                    if pk.get(w.id, -1) >= w.wait_value:
                        implied = True
                        break
            if implied:
                kept.remove(w)
                n_pruned += 1
        if len(kept) < len(waits):
            inst.sync_info = mybir.SyncInfo(
                on_wait=kept, on_update=list(si.on_update))
    return n_pruned


_NO_SPLIT_TYPES = {
    "InstNoOp", "InstEventSemaphore", "InstAllEngineBarrier",
    "InstBranchHint", "InstHalt", "InstCall", "InstOverlayCall",
    "InstCompareAndBranch", "InstIndirectBranch",
}


def _split_dma_waits(nc, nopsem, max_waits=1):
    """The 64-byte ISA instruction encodings carry a single sync-wait slot.
    Move excess waits onto a same-engine nofuse-nop inserted right before the
    instruction — the sequencer executes the nop's waits before dispatching
    it, preserving the happens-before order."""
    n_split = 0
    for f in nc.m.functions:
        for bb in f.blocks:
            il = bb.instructions
            i = 0
            while i < len(il):
                inst = il[i]
                tn = type(inst).__name__
                si = inst.sync_info
                if (tn not in _NO_SPLIT_TYPES and si and si.on_wait
                        and len(si.on_wait) > max_waits):
                    waits = list(si.on_wait)
                    keep = waits[-max_waits:]
                    extra = waits[:-max_waits]
                    for j, w in enumerate(extra):
                        nop = mybir.InstNoOp(
                            name=f"{inst.name}-waitnop{j}", ins=[], outs=[])
                        nop.engine = inst.engine
                        nop.bass_nofuse = True
                        upd = mybir.SyncUpdate(
                            sync_type="semaphore", id=nopsem.num,
                            ant_name="nopsem", update_mode="sem-inc",
                            update_value=1, update_reg=None)
                        nop.sync_info = mybir.SyncInfo(
                            on_wait=[w], on_update=[upd])
                        il.insert(i, nop)
                        i += 1
                        n_split += 1
                    inst.sync_info = mybir.SyncInfo(
                        on_wait=keep, on_update=list(si.on_update))
                i += 1
    return n_split


def build_nc(t=T, dbg=False):
    """Build the single-core SPMD graph (same for all 8 cores)."""
    nblk = t // 128
    n = t * B
    nc = bass.Bass()
    nopsem = nc.alloc_semaphore("nopsem")

    # ---- inputs ----
    xT_d = [nc.declare_dram_parameter(f"xT{b}", [D, t], F32, isOutput=False)
            for b in range(B)]
    posT_d = nc.declare_dram_parameter("posT", [D, t], F32, isOutput=False)
    # weight slices pre-permuted to [Dc, o, f] (f innermost) and flattened
    wq_s = nc.declare_dram_parameter("wq_s", [D, DH * F], F32, isOutput=False)
    wk_s = nc.declare_dram_parameter("wk_s", [D, DH * F], F32, isOutput=False)
    wp_s = nc.declare_dram_parameter("wp_s", [D, DH * F], F32, isOutput=False)
    wv_s = nc.declare_dram_parameter("wv_s", [D, DH * F], BF16, isOutput=False)
    # out-proj slice pre-permuted to [dh, Do, f]
    wo_s = nc.declare_dram_parameter("wo_s", [DH, D * F], BF16, isOutput=False)
    factor_in = nc.declare_dram_parameter("factor", [128, F], F32, isOutput=False)
    bqrw8 = nc.declare_dram_parameter("bqrw8", [DH, 1], F32, isOutput=False)
    bqrr8 = nc.declare_dram_parameter("bqrr8", [DH, 1], F32, isOutput=False)
    bk_in = nc.declare_dram_parameter("bk", [DH, 1], F32, isOutput=False)
    bv_in = nc.declare_dram_parameter("bv", [DH, 1], F32, isOutput=False)
    br_in = nc.declare_dram_parameter("br", [DH, 1], F32, isOutput=False)
    bout8 = nc.declare_dram_parameter("bout8", [1, D], F32, isOutput=False)
    id_in = nc.declare_dram_parameter("id128", [128, 128], F32, isOutput=False)
    idb_in = nc.declare_dram_parameter("id128b", [128, 128], BF16, isOutput=False)
    out_ext = nc.declare_dram_parameter("out", [n, D], F32, isOutput=True)
    if dbg:
        dbg_band = nc.declare_dram_parameter("dbg_band", [128, t + 128], F32,
                                             isOutput=True)
        dbg_score = nc.declare_dram_parameter("dbg_score", [128, 2048], F32,
                                              isOutput=True)
        dbg_probs = nc.declare_dram_parameter("dbg_probs", [128, 2048], BF16,
                                              isOutput=True)
        dbg_stat = nc.declare_dram_parameter("dbg_stat", [128, 2], F32,
                                             isOutput=True)
        dbg_v = nc.declare_dram_parameter("dbg_v", [128, (t // 128) * DH], BF16,
                                          isOutput=True)
        dbg_ctx = nc.declare_dram_parameter("dbg_ctx", [128, DH], BF16,
                                            isOutput=True)
        dbg_pt = nc.declare_dram_parameter("dbg_pt", [128, 128], BF16,
                                           isOutput=True)

    wband = t + 128  # band buffer free size (data width + pad)

    with tile.TileContext(nc) as tc:
        import contextlib

        es = contextlib.ExitStack()
        with es:
            # persistent SBUF tensors (bufs=1 pool, one tag each)
            pers = es.enter_context(tc.tile_pool(name="pers", bufs=1))

            def sb(name, shape, dt):
                return pers.tile(shape, dt, tag=name, name=name)

            # xT working set, shared across b phases and pos phase
            xT = [sb(f"xT_{c}", [128, t], F32) for c in range(4)]
            # batch-b operands live at partition base 64*b on both matmul
            # sides (matmul requires equal base partitions)
            qrwP = sb("qrwP", [128, t], F32)
            qrrP = sb("qrrP", [128, t], F32)
            kT_pack = sb("kT_pack", [128, t], F32)
            rT2 = sb("rT2", [128, t], F32)
            vT_sb2 = [sb(f"vT_sb{b}", [DH, t], BF16) for b in range(B)]
            v_sb = [sb(f"v_{b}", [128, nblk * DH], BF16) for b in range(B)]
            wt_q = sb("wt_q", [128, 4 * DH], F32)
            wt_k = sb("wt_k", [128, 4 * DH], F32)
            wt_v = sb("wt_v", [128, 4 * DH], F32)
            wt_p = sb("wt_p", [128, 4 * DH], F32)
            wt_o = sb("wt_o", [DH, D], BF16)
            band = [sb(f"band_{i}", [128, wband], F32) for i in range(2)]
            fct = sb("fct", [128, F], F32)
            fct_bf = sb("fct_bf", [128, F], BF16)
            b_qrw = sb("b_qrw", [DH, 1], F32)
            b_qrr = sb("b_qrr", [DH, 1], F32)
            b_k = sb("b_k", [DH, 1], F32)
            b_v = sb("b_v", [DH, 1], F32)
            b_r = sb("b_r", [DH, 1], F32)
            b_o = sb("b_o", [1, D], F32)
            id128 = sb("id128_sb", [128, 128], F32)
            id128b = sb("id128b_sb", [128, 128], BF16)
            ones1 = sb("ones1", [1, 128], F32)

            syn_in = es.enter_context(tc.tile_pool(name="syn_in", bufs=6))
            syn_pr = es.enter_context(tc.tile_pool(name="syn_pr", bufs=2))
            syn_f32 = es.enter_context(tc.tile_pool(name="syn_f32", bufs=2))
            mm_ps = es.enter_context(tc.tile_pool(name="mm_ps", bufs=2, space="PSUM"))
            score_ps = es.enter_context(
                tc.tile_pool(name="score_ps", bufs=1, space="PSUM"))
            pv_ps = es.enter_context(tc.tile_pool(name="pv_ps", bufs=1, space="PSUM"))
            probs_pool = es.enter_context(tc.tile_pool(name="probs", bufs=2))
            bdsh_pool = es.enter_context(tc.tile_pool(name="bdsh", bufs=12))
            pt_pool = es.enter_context(tc.tile_pool(name="pt", bufs=4))
            stat_pool = es.enter_context(tc.tile_pool(name="stat", bufs=4))
            ctx_pool = es.enter_context(tc.tile_pool(name="ctx", bufs=4))
            oev_pool = es.enter_context(tc.tile_pool(name="oev", bufs=6))

            def rawap(tl, off, pattern):
                a = tl[:]
                return bass.AP(a.tensor, a.offset + off, pattern)

            # ---- small loads ----
            nc.sync.dma_start(fct[:], factor_in[:])
            nc.vector.tensor_copy(fct_bf[:], fct[:])
            nc.sync.dma_start(b_qrw[:], bqrw8[:])
            nc.sync.dma_start(b_qrr[:], bqrr8[:])
            nc.sync.dma_start(b_k[:], bk_in[:])
            nc.sync.dma_start(b_v[:], bv_in[:])
            nc.sync.dma_start(b_r[:], br_in[:])
            nc.sync.dma_start(b_o[:], bout8[:])
            nc.sync.dma_start(id128[:], id_in[:])
            nc.sync.dma_start(id128b[:], idb_in[:])
            nc.vector.memset(ones1[:], 1.0)

            # ---- weight synthesis: Wt[Dc, o] = sum_f w[Dc, o, f] * factor[f]
            fbc = rawap(fct, 0, [[F, 128], [0, DH // 2], [1, F]])
            fbc_bf = rawap(fct_bf, 0, [[F, 128], [0, DH // 2], [1, F]])

            HO = DH // 2  # o-values per synthesis half-tile

            def synth(w_dram, wt_dst, dt, fb):
                for c in range(4):
                    for hh in range(2):
                        win = syn_in.tile([128, HO * F], dt, tag=f"syn_in{dt}",
                                          name=f"win_{w_dram.name}_{c}_{hh}")
                        nc.sync.dma_start(
                            win[:],
                            w_dram[128 * c:128 * (c + 1),
                                   HO * F * hh:HO * F * (hh + 1)])
                        pr = syn_pr.tile([128, HO * F], dt, tag=f"syn_pr{dt}",
                                         name=f"pr_{w_dram.name}_{c}_{hh}")
                        nc.vector.tensor_tensor(
                            pr[:].rearrange("p (o f) -> p o f", f=F),
                            win[:].rearrange("p (o f) -> p o f", f=F),
                            fb, op=ALU.mult)
                        red = syn_f32.tile([128, HO], F32, tag="syn_f32",
                                           name=f"red_{w_dram.name}_{c}_{hh}")
                        nc.vector.tensor_reduce(
                            red[:], pr[:].rearrange("p (o f) -> p o f", f=F),
                            axis=mybir.AxisListType.X, op=ALU.add)
                        nc.vector.tensor_copy(
                            wt_dst[:, DH * c + HO * hh:DH * c + HO * (hh + 1)],
                            red[:])

            synth(wq_s, wt_q, F32, fbc)
            synth(wk_s, wt_k, F32, fbc)
            synth(wv_s, wt_v, BF16, fbc_bf)
            synth(wp_s, wt_p, F32, fbc)

            # out-proj synthesis in 16 chunks (bf16)
            fbc_o = rawap(fct_bf, 0, [[F, DH], [0, HO], [1, F]])
            for c8 in range(16):
                wo_in = syn_in.tile([DH, HO * F], BF16, tag=f"syn_in{BF16}",
                                    name=f"wo_in_{c8}")
                nc.sync.dma_start(
                    wo_in[:], wo_s[:, HO * F * c8:HO * F * (c8 + 1)])
                wo_pr = syn_pr.tile([DH, HO * F], BF16, tag=f"syn_pr{BF16}",
                                    name=f"wo_pr_{c8}")
                nc.vector.tensor_tensor(
                    wo_pr[:].rearrange("p (o f) -> p o f", f=F),
                    wo_in[:].rearrange("p (o f) -> p o f", f=F),
                    fbc_o, op=ALU.mult)
                wo_red = syn_f32.tile([DH, HO], F32, tag="syn_f32",
                                      name=f"wo_red_{c8}")
                nc.vector.tensor_reduce(
                    wo_red[:], wo_pr[:].rearrange("p (o f) -> p o f", f=F),
                    axis=mybir.AxisListType.X, op=ALU.add)
                nc.vector.tensor_copy(wt_o[:, HO * c8:HO * (c8 + 1)], wo_red[:])

            # ---- projections (xT slabs shared across phases) ----
            nstr_t = t // 512

            def load_xt(src_dram):
                for c in range(4):
                    for pp in range(4):
                        q = t // 4
                        nc.sync.dma_start(
                            xT[c][:, q * pp:q * (pp + 1)],
                            src_dram[128 * c:128 * (c + 1),
                                     q * pp:q * (pp + 1)])

            def project(wt, evacs):
                # evacs: list of (dst, row0, bias, scale)
                for s in range(nstr_t):
                    ps = mm_ps.tile([DH, 512], F32, tag="mm",
                                    name=f"proj_{wt.name}_{s}")
                    for c in range(4):
                        nc.tensor.matmul(
                            ps[:], wt[:, DH * c:DH * (c + 1)],
                            xT[c][:, 512 * s:512 * (s + 1)],
                            start=(c == 0), stop=(c == 3))
                    for dst, row0, bia, sc in evacs:
                        nc.scalar.activation(
                            dst[row0:row0 + DH, 512 * s:512 * (s + 1)], ps[:],
                            AF.Identity, bias=bia[:, 0:1], scale=sc)

            for b in range(B):
                load_xt(xT_d[b])
                project(wt_q, [(qrwP, DH * b, b_qrw, SCALE),
                               (qrrP, DH * b, b_qrr, SCALE)])
                project(wt_k, [(kT_pack, DH * b, b_k, 1.0)])
                project(wt_v, [(vT_sb2[b], 0, b_v, 1.0)])
                for tt in range(nblk):
                    vp = mm_ps.tile([128, DH], BF16, tag="mmt", bufs=1,
                                    name=f"vp_{b}_{tt}")
                    nc.tensor.transpose(
                        vp[:], vT_sb2[b][:, 128 * tt:128 * (tt + 1)],
                        id128b[0:DH, 0:DH])
                    nc.vector.tensor_copy(
                        v_sb[b][:, DH * tt:DH * (tt + 1)], vp[:])
            load_xt(posT_d)
            project(wt_p, [(rT2, 0, b_r, 1.0), (rT2, DH, b_r, 1.0)])

            # ---- attention ----
            for b in range(B):
                for blk in range(nblk):
                    i0 = 128 * blk
                    w = i0 + 128          # causal row width
                    nstr = (w + 511) // 512
                    jr0 = t - 128 - i0    # first needed rel-pos column
                    bnd = band[blk % 2]

                    sc_ps = score_ps.tile([128, 2048], F32, tag="score",
                                          name=f"sc_{b}_{blk}")
                    # AC strips
                    for s in range(nstr):
                        ws = min(512, w - 512 * s)
                        nc.tensor.matmul(
                            sc_ps[:, 512 * s:512 * s + ws],
                            qrwP[DH * b:DH * b + DH, i0:i0 + 128],
                            kT_pack[DH * b:DH * b + DH, 512 * s:512 * s + ws],
                            start=True, stop=False)
                    # BD strips -> band
                    for s in range(nstr):
                        ws = min(512, w - 512 * s)
                        bd = mm_ps.tile([128, 512], F32, tag="mm",
                                        name=f"bd_{b}_{blk}_{s}")
                        nc.tensor.matmul(
                            bd[:, :ws], qrrP[DH * b:DH * b + DH, i0:i0 + 128],
                            rT2[DH * b:DH * b + DH,
                                jr0 + 512 * s:jr0 + 512 * s + ws],
                            start=True, stop=True)
                        nc.scalar.copy(bnd[:, 512 * s:512 * s + ws], bd[:, :ws])
                    # pad supplies causal mask of the diagonal tile
                    nc.vector.memset(bnd[:, w:w + 127], NEG_BIG)
                    # rel-shift extraction + inject into score PSUM
                    for s in range(nstr):
                        ws = min(512, w - 512 * s)
                        bsh = bdsh_pool.tile([128, 512], F32, tag="bdsh",
                                             name=f"bsh_{b}_{blk}_{s}")
                        nt = ws // 128
                        diag = rawap(
                            bnd, 127 + 512 * s,
                            [[wband - 1, 128], [128, nt], [1, 128]])
                        dstap = rawap(
                            bsh, 0, [[512, 128], [128, nt], [1, 128]])
                        nc.sync.dma_start(dstap, diag)
                        nc.tensor.matmul(
                            sc_ps[:, 512 * s:512 * s + ws], id128[:],
                            bsh[:, :ws], start=False, stop=True)
                    if dbg and b == 0 and blk == DBG_BLK:
                        nc.sync.dma_start(dbg_band[:, :w + 127],
                                          bnd[:, :w + 127])
                        dbg_sc_sb = pers.tile([128, 2048], F32, tag="dbg_sc",
                                              name="dbg_sc")
                        nc.vector.tensor_copy(dbg_sc_sb[:, :w], sc_ps[:, :w])
                        nc.sync.dma_start(dbg_score[:, :w], dbg_sc_sb[:, :w])
                    # softmax
                    negmax = stat_pool.tile([128, 1], F32, tag="negmax",
                                            name=f"negmax_{b}_{blk}")
                    nc.vector.tensor_reduce(
                        negmax[:], sc_ps[:, :w], axis=mybir.AxisListType.X,
                        op=ALU.max, negate=True)
                    rowsum = stat_pool.tile([128, 1], F32, tag="rowsum",
                                            name=f"rowsum_{b}_{blk}")
                    probs = probs_pool.tile([128, 2048], BF16, tag="probs",
                                            name=f"probs_{b}_{blk}")
                    nc.scalar.activation(
                        probs[:, :w], sc_ps[:, :w], AF.Exp,
                        bias=negmax[:, 0:1], scale=1.0,
                        accum_out=rowsum[:, 0:1])
                    if dbg and b == 0 and blk == DBG_BLK:
                        nc.sync.dma_start(dbg_probs[:, :w], probs[:, :w])
                        nc.sync.dma_start(dbg_stat[:, 0:1], negmax[:])
                        nc.sync.dma_start(dbg_stat[:, 1:2], rowsum[:])
                    recip = stat_pool.tile([128, 1], F32, tag="recip",
                                           name=f"recip_{b}_{blk}")
                    nc.vector.reciprocal(recip[:], rowsum[:])
                    # PV
                    pv = pv_ps.tile([128, DH], F32, tag="pv",
                                    name=f"pv_{b}_{blk}")
                    for tt in range(blk + 1):
                        ptp = mm_ps.tile([128, 128], BF16, tag="mmt", bufs=1,
                                         name=f"ptp_{b}_{blk}_{tt}")
                        nc.tensor.transpose(
                            ptp[:], probs[:, 128 * tt:128 * (tt + 1)],
                            id128b[:])
                        pT = pt_pool.tile([128, 128], BF16, tag="pt",
                                          name=f"pt_{b}_{blk}_{tt}")
                        nc.vector.tensor_copy(pT[:], ptp[:])
                        if dbg and b == 0 and blk == DBG_BLK and tt == 1:
                            nc.sync.dma_start(dbg_pt[:], pT[:])
                        nc.tensor.matmul(
                            pv[:], pT[:], v_sb[b][:, DH * tt:DH * (tt + 1)],
                            start=(tt == 0), stop=(tt == blk))
                    ctx = ctx_pool.tile([128, DH], BF16, tag="ctx",
                                        name=f"ctx_{b}_{blk}")
                    nc.vector.tensor_scalar_mul(
                        ctx[:], pv[:], recip[:, 0:1])
                    if dbg and b == 0 and blk == DBG_BLK:
                        nc.sync.dma_start(dbg_ctx[:], ctx[:])
                        nc.sync.dma_start(dbg_v[:], v_sb[0][:])
                    ctp = mm_ps.tile([DH, 128], BF16, tag="mmt", bufs=1,
                                     name=f"ctp_{b}_{blk}")
                    nc.tensor.transpose(ctp[:], ctx[:], id128b[:])
                    ctxT = ctx_pool.tile([DH, 128], BF16, tag="ctxT",
                                         name=f"ctxT_{b}_{blk}")
                    nc.vector.tensor_copy(ctxT[:], ctp[:])
                    # out-proj partial for this query block
                    op = mm_ps.tile([128, D], F32, tag="mm",
                                    name=f"op_{b}_{blk}")
                    nc.tensor.matmul(op[:], ctxT[:], wt_o[:],
                                     start=True, stop=False)
                    nc.tensor.matmul(op[:], ones1[:], b_o[:],
                                     start=False, stop=True)
                    oev = oev_pool.tile([128, D], F32, tag="oev",
                                        name=f"oev_{b}_{blk}")
                    nc.vector.tensor_copy(oev[:], op[:])
                    dst = bass.AP(out_ext, (b * t + i0) * D, [[D, 128], [1, D]])
                    nc.sync.dma_start(dst, oev[:])

    _prune_dma_waits(nc)
    _split_dma_waits(nc, nopsem)
    return nc


# ---------------- host side ----------------

def _prep_core_inputs(h, x, pos, factor, in_proj_weight, in_proj_bias,
                      pos_proj_weight, pos_proj_bias, out_proj_weight,
                      out_proj_bias, r_w_bias, r_r_bias, t=T):
    bf = ml_dtypes.bfloat16
    sl = slice(DH * h, DH * (h + 1))

    def perm(w, dt):  # [o, Dc, f] -> [Dc, o, f] flattened
        return np.ascontiguousarray(np.transpose(w, (1, 0, 2))).reshape(
            D, -1).astype(dt)

    wq = perm(in_proj_weight[sl], np.float32)
    wk = perm(in_proj_weight[D + DH * h:D + DH * (h + 1)], np.float32)
    wv = perm(in_proj_weight[2 * D + DH * h:2 * D + DH * (h + 1)], bf)
    wp = perm(pos_proj_weight[sl], np.float32)
    # out-proj column slice: [Do, dh, f] -> [dh, Do, f]
    wo = np.ascontiguousarray(
        np.transpose(out_proj_weight[:, sl, :], (1, 0, 2))).reshape(
        DH, -1).astype(bf)

    b_q = in_proj_bias[sl] @ factor
    b_k = in_proj_bias[D + DH * h:D + DH * (h + 1)] @ factor
    b_v = in_proj_bias[2 * D + DH * h:2 * D + DH * (h + 1)] @ factor
    b_r = (pos_proj_bias @ factor)[sl]
    b_o = out_proj_bias @ factor
    rw = r_w_bias[h] @ factor
    rr = r_r_bias[h] @ factor

    d = {
        "posT": np.ascontiguousarray(pos.reshape(t, D).T).astype(np.float32),
        "wq_s": wq, "wk_s": wk, "wv_s": wv, "wp_s": wp, "wo_s": wo,
        "factor": np.tile(factor.reshape(1, F), (128, 1)).astype(np.float32),
        "bqrw8": ((b_q + rw) * SCALE).reshape(DH, 1).astype(np.float32),
        "bqrr8": ((b_q + rr) * SCALE).reshape(DH, 1).astype(np.float32),
        "bk": b_k.reshape(DH, 1).astype(np.float32),
        "bv": b_v.reshape(DH, 1).astype(np.float32),
        "br": b_r.reshape(DH, 1).astype(np.float32),
        "bout8": (b_o / H).reshape(1, D).astype(np.float32),
        "id128": np.eye(128, dtype=np.float32),
        "id128b": np.eye(128, dtype=bf),
    }
    for b in range(B):
        d[f"xT{b}"] = np.ascontiguousarray(x[:, b, :].T).astype(np.float32)
    return d


def make_in_maps(inputs, t=T):
    x = np.asarray(inputs["x"], np.float32)
    pos = np.asarray(inputs["pos"], np.float32)
    factor = (np.asarray(inputs["fmap_w"], np.float32)
              @ np.asarray(inputs["factor_in"], np.float32)
              + np.asarray(inputs["fmap_b"], np.float32))
    args = dict(
        x=x, pos=pos, factor=factor,
        in_proj_weight=np.asarray(inputs["in_proj_weight"], np.float32),
        in_proj_bias=np.asarray(inputs["in_proj_bias"], np.float32),
        pos_proj_weight=np.asarray(inputs["pos_proj_weight"], np.float32),
        pos_proj_bias=np.asarray(inputs["pos_proj_bias"], np.float32),
        out_proj_weight=np.asarray(inputs["out_proj_weight"], np.float32),
        out_proj_bias=np.asarray(inputs["out_proj_bias"], np.float32),
        r_w_bias=np.asarray(inputs["r_w_bias"], np.float32),
        r_r_bias=np.asarray(inputs["r_r_bias"], np.float32),
    )
    return [_prep_core_inputs(h, t=t, **args) for h in range(H)]


def kernel(**inputs):
    t = T
    in_maps = make_in_maps(inputs, t=t)
    nc = build_nc(t=t)
    res = run_bass_kernel_spmd(nc, in_maps, core_ids=list(range(H)))
    out = np.zeros((B, t, D), np.float64)
    for r in res.results:
        out += np.asarray(r["out"], np.float32).astype(np.float64).reshape(B, t, D)
    return np.ascontiguousarray(out.transpose(1, 0, 2)).astype(np.float32)
